# revision 1
# baseline (speedup 1.0000x reference)
"""Trainium2 Bass kernel for nn_ChEBIRecNN (gnn_message_passing).

Strategy
--------
D=256 DAGs sharded 32/core across 8 NeuronCores (data parallel).

The per-level softmax-attention gather is reformulated with predecessor
COUNT matrices (host-precomputed from pred_idx):
    C_d[j,k'] = #{p : pred_idx[d,l,k',p] == j}
    den[k',f] = sum_j C[k',j] * E[j,f],   E = exp(att*out)
    num[k',f] = sum_j C[k',j] * (E*y)[j,f]
    agg       = num / den
turning gather+softmax+reduce into two dense 64-contraction matmuls per
DAG, batched 2 DAGs/tile via block-diagonal count matrices.

State y^T = (att*out)/16 kept in fp16 B-layout [104(f) x 128(2 dags x 64
nodes)] tiles; att_w and the /16 scaling are folded into the weights on
the host. atom_feats are pre-transposed/cast to fp16 on the host (with a
ones-row so biases fold into the same matmul).

Per level (all 16 pair-tiles):  PE transpose y^T -> y_A, ACT exp(16*x),
DVE E*y, PE count-matmuls (blockdiag moving), DVE divide, PE merge+atoms
matmuls, ACT relu -> next y^T.

Final sink softmax-pool: per-core partial sums (sum e^{g*s}*s, sum
e^{g*s}) are computed on-device and reduced across cores on the host,
followed by the tiny [104]x[104,500] output linear.
"""

import sys

sys.path.insert(0, "/opt/trn_rl_repo")

import numpy as np

import concourse.bacc as bacc
import concourse.bass as bass
import concourse.mybir as mybir
import concourse.tile as tile
from concourse.bass_utils import run_bass_kernel_spmd

D, L, K, P, F, C = 256, 64, 64, 8, 104, 500
NCORES = 8
DPC = D // NCORES          # 32 dags per core
NPAIR = DPC // 2           # 16 pair-tiles
SCALE = 16.0               # state stored as y/16 (fp16 headroom for E*y)

F16 = mybir.dt.float16
F32 = mybir.dt.float32

_compiled = {}


def _host_prep(atom_feats, pred_idx, W1, b1, Wm, bm, att_w, dag_w):
    """Build per-core DMA-ready tensors (numpy only)."""
    att = att_w.astype(np.float64)
    # effective weights (att folding + 1/SCALE state scaling), see module doc
    w1_eff = (W1.astype(np.float64) * att[None, :] / SCALE).astype(np.float16)
    b1_eff = (b1.astype(np.float64) * att / SCALE).astype(np.float16)
    wtop = (Wm[:F].astype(np.float64) * att[None, :] / att[:, None]).astype(np.float16)
    wbot = (Wm[F:].astype(np.float64) * att[None, :] / SCALE).astype(np.float16)
    bm_eff = (bm.astype(np.float64) * att / SCALE).astype(np.float16)


    ident = np.eye(F, dtype=np.float16)                            # [104,104]
    # final-pool exp scale: exp(dag_w * sink_true) = exp(y_stored * dag_w*16/att)
    dscale = (dag_w.astype(np.float64) * SCALE / att).astype(np.float32)[:, None]

    # count matrices: CT[d,l,j,k'] = #{p: pred_idx[d,l,k',p]==j}
    rows = np.arange(D * (L - 1) * K, dtype=np.int64).repeat(P) * K
    lin = rows + pred_idx.reshape(-1).astype(np.int64)
    ct = np.bincount(lin, minlength=D * (L - 1) * K * K).astype(np.float16)
    ct = ct.reshape(D, L - 1, K, K)                                # [d,l,j?,k?]
    # ct[d,l,k',j] counted as [row=k', col=j]; we need moving[j,k'] -> transpose
    ct = np.swapaxes(ct, 2, 3)                                     # [d,l,j,k']

    # atomsT: [core, level, 105, NPAIR*128] fp16, row 104 = ones
    at = np.swapaxes(atom_feats, 2, 3).astype(np.float16)          # [d,l,f,k]
    at = at.reshape(NCORES, DPC, L, F, K)

    per_core = []
    for c in range(NCORES):
        a = at[c]                                                  # [32,64,104,64]
        a = a.reshape(NPAIR, 2, L, F, K)
        # [level, f, pair, dag-in-pair, k] -> [level, f, pair*128]
        a = a.transpose(2, 3, 0, 1, 4).reshape(L, F, NPAIR * 2 * K)
        atomsT = np.ascontiguousarray(a)                           # [64,104,2048]

        cc = ct.reshape(NCORES, DPC, L - 1, K, K)[c]               # [32,63,64,64]
        cc = cc.reshape(NPAIR, 2, L - 1, K, K)
        # full-width blockdiag halves (zeros baked in) so the per-level DMA
        # is a single contiguous 2D copy per half
        c_even = np.zeros((L - 1, K, NPAIR, 2 * K), np.float16)
        c_even[:, :, :, 0:K] = cc[:, 0].transpose(1, 2, 0, 3)
        c_even = np.ascontiguousarray(c_even.reshape(L - 1, K, NPAIR * 2 * K))
        c_odd = np.zeros((L - 1, K, NPAIR, 2 * K), np.float16)
        c_odd[:, :, :, K:2 * K] = cc[:, 1].transpose(1, 2, 0, 3)
        c_odd = np.ascontiguousarray(c_odd.reshape(L - 1, K, NPAIR * 2 * K))
        per_core.append({
            "atomsT": atomsT, "c_even": c_even, "c_odd": c_odd,
            "w1": w1_eff, "wbot": np.ascontiguousarray(wbot),
            "wtop": np.ascontiguousarray(wtop),
            "b1v": b1_eff.astype(np.float32)[:, None],
            "bmv": bm_eff.astype(np.float32)[:, None],
            "ident": ident, "dscale": dscale,
        })
    return per_core


def _final(nc, pool, y_tiles, d_out):
    """Emit raw sink state (y/16 at node K-1) [F, DPC]; host finishes the
    tiny softmax-pool + output linear."""
    sk = pool.tile([F, DPC], F32, tag="sk")
    for t in range(NPAIR):
        # sink columns: node K-1 of each dag in the pair
        nc.scalar.copy(sk[:, 2 * t:2 * t + 2],
                       y_tiles[t][:].rearrange("p (d k) -> p d k", k=K)[:, :, K - 1])
    nc.sync.dma_start(d_out, sk[:])


def _build_program(levels=L, skip_final=False):
    nc = bacc.Bacc("TRN2", target_bir_lowering=False, debug=False,
                   num_devices=NCORES)

    d_atomsT = nc.dram_tensor("atomsT", [L, F, NPAIR * 128], F16,
                              kind="ExternalInput").ap()
    d_ceven = nc.dram_tensor("c_even", [L - 1, K, NPAIR * 128], F16,
                             kind="ExternalInput").ap()
    d_codd = nc.dram_tensor("c_odd", [L - 1, K, NPAIR * 128], F16,
                            kind="ExternalInput").ap()
    d_w1 = nc.dram_tensor("w1", [F, F], F16, kind="ExternalInput").ap()
    d_wbot = nc.dram_tensor("wbot", [F, F], F16, kind="ExternalInput").ap()
    d_b1v = nc.dram_tensor("b1v", [F, 1], F32, kind="ExternalInput").ap()
    d_bmv = nc.dram_tensor("bmv", [F, 1], F32, kind="ExternalInput").ap()
    d_wtop = nc.dram_tensor("wtop", [F, F], F16, kind="ExternalInput").ap()
    d_ident = nc.dram_tensor("ident", [F, F], F16, kind="ExternalInput").ap()
    d_dscale = nc.dram_tensor("dscale", [F, 1], F32, kind="ExternalInput").ap()
    d_out = nc.dram_tensor("sinks", [F, DPC], F32, kind="ExternalOutput").ap()

    with tile.TileContext(nc) as tc:
        with tc.tile_pool(name="pool", bufs=1) as pool, \
             tc.tile_pool(name="psum", space="PSUM", bufs=1) as psum:
            # constants / weights
            w1 = pool.tile([F, F], F16, tag="w1")
            wbot = pool.tile([F, F], F16, tag="wbot")
            wtop = pool.tile([F, F], F16, tag="wtop")
            ident = pool.tile([F, F], F16, tag="ident")
            dscale = pool.tile([F, 1], F32, tag="dscale")
            b1v = pool.tile([F, 1], F32, tag="b1v")
            bmv = pool.tile([F, 1], F32, tag="bmv")
            nc.sync.dma_start(w1[:], d_w1)
            nc.sync.dma_start(wbot[:], d_wbot)
            nc.sync.dma_start(wtop[:], d_wtop)
            nc.sync.dma_start(ident[:], d_ident)
            nc.sync.dma_start(dscale[:], d_dscale)
            nc.sync.dma_start(b1v[:], d_b1v)
            nc.sync.dma_start(bmv[:], d_bmv)

            y_tiles = [None] * NPAIR

            def atoms_tile(level):
                a = pool.tile([F, NPAIR * 128], F16, tag="atoms", bufs=3)
                nc.sync.dma_start(a[:], d_atomsT[level])
                return a

            # ---- level 0: y0 = relu(atoms0 @ W1_aug) ----
            a0 = atoms_tile(0)
            for t in range(NPAIR):
                z = psum.tile([F, 128], F32, tag="z", bufs=3)
                nc.tensor.matmul(z[:], w1[:], a0[:, 128 * t:128 * (t + 1)],
                                 start=True, stop=True)
                y = pool.tile([F, 128], F16, tag=f"y{t}", bufs=2)
                nc.scalar.activation(y[:], z[:],
                                     mybir.ActivationFunctionType.Relu,
                                     bias=b1v[:])
                y_tiles[t] = y

            # ---- levels 1..63 ----
            for lvl in range(1, levels):
                cb = pool.tile([128, NPAIR * 128], F16, tag="cbuf", bufs=3,
                               name="cb")
                nc.sync.dma_start(cb[0:K, :], d_ceven[lvl - 1])
                nc.sync.dma_start(cb[K:128, :], d_codd[lvl - 1])
                al = atoms_tile(lvl)
                for t in range(NPAIR):
                    ya = psum.tile([128, F], F16, tag="ya", bufs=2)
                    nc.tensor.matmul(ya[:], y_tiles[t][:], ident[:],
                                     is_transpose=True)
                    e = pool.tile([128, 2 * F], F16, tag="e", bufs=3)
                    nc.scalar.activation(e[:, 0:F], ya[:],
                                         mybir.ActivationFunctionType.Exp,
                                         scale=SCALE)
                    nc.vector.tensor_tensor(e[:, F:2 * F], e[:, 0:F], ya[:],
                                            op=mybir.AluOpType.mult)
                    dn = psum.tile([F, 256], F32, tag="dn", bufs=3)
                    cslice = cb[:, 128 * t:128 * (t + 1)]
                    nc.tensor.matmul(dn[:, 0:128], e[:, 0:F], cslice,
                                     start=True, stop=True)
                    nc.tensor.matmul(dn[:, 128:256], e[:, F:2 * F], cslice,
                                     start=True, stop=True)
                    rd = pool.tile([F, 128], F32, tag="rd", bufs=3)
                    nc.vector.reciprocal(rd[:], dn[:, 0:128])
                    ag = pool.tile([F, 128], F16, tag="ag", bufs=3)
                    nc.vector.tensor_tensor(ag[:], dn[:, 128:256], rd[:],
                                            op=mybir.AluOpType.mult)
                    z = psum.tile([F, 128], F32, tag="z", bufs=3)
                    nc.tensor.matmul(z[:], wtop[:], ag[:],
                                     start=True, stop=False)
                    nc.tensor.matmul(z[:], wbot[:],
                                     al[:, 128 * t:128 * (t + 1)],
                                     start=False, stop=True)
                    y = pool.tile([F, 128], F16, tag=f"y{t}", bufs=2)
                    nc.scalar.activation(y[:], z[:],
                                         mybir.ActivationFunctionType.Relu,
                                         bias=bmv[:])
                    y_tiles[t] = y

            # ---- final: per-core partial softmax-pool over local dags ----
            if skip_final:
                pn = pool.tile([F, DPC], F32, tag="pn")
                nc.scalar.copy(pn[:], y_tiles[0][:, 0:DPC])
                nc.sync.dma_start(d_out, pn[:])
            else:
                _final(nc, pool, y_tiles, d_out)

    nc.compile()
    return nc


def kernel(atom_feats, pred_idx, W1, b1, Wm, bm, att_w, dag_w, Wf, bf):
    atom_feats = np.asarray(atom_feats, np.float32)
    pred_idx = np.asarray(pred_idx, np.int32)
    per_core = _host_prep(atom_feats, pred_idx,
                          np.asarray(W1, np.float32), np.asarray(b1, np.float32),
                          np.asarray(Wm, np.float32), np.asarray(bm, np.float32),
                          np.asarray(att_w, np.float32), np.asarray(dag_w, np.float32))

    if "nc" not in _compiled:
        _compiled["nc"] = _build_program()
    nc = _compiled["nc"]

    import os
    in_maps = [{k: v for k, v in pc.items()} for pc in per_core]
    trace = bool(os.environ.get("BASS_KERNEL_TRACE"))
    res = run_bass_kernel_spmd(nc, in_maps, list(range(NCORES)), trace=trace)
    _compiled["exec_time_ns"] = res.exec_time_ns

    att = np.asarray(att_w, np.float64)[:, None]
    dagw = np.asarray(dag_w, np.float64)[:, None]
    sinks = np.concatenate(
        [np.asarray(r["sinks"], np.float64) for r in res.results], axis=1)
    sink = sinks * SCALE / att                     # [F, D] true sink values
    u = np.exp(dagw * sink)
    pooled = (u * sink).sum(1) / u.sum(1)
    out = pooled @ np.asarray(Wf, np.float64) + np.asarray(bf, np.float64)
    return out.astype(np.float32)



# revision 5
# speedup vs baseline: 1.0276x; 1.0276x over previous
"""Trainium2 Bass kernel for nn_ChEBIRecNN (gnn_message_passing).

Strategy
--------
D=256 DAGs sharded 32/core across 8 NeuronCores (data parallel).

The per-level softmax-attention gather is reformulated with predecessor
COUNT matrices (host-precomputed from pred_idx):
    C_d[j,k'] = #{p : pred_idx[d,l,k',p] == j}
    den[k',f] = sum_j C[j,k'] * E[j,f],   E = exp(att*out)
    num[k',f] = sum_j C[j,k'] * (E*y)[j,f]
    agg       = num / den
turning gather+softmax+reduce into dense 128-contraction matmuls with
the block-diagonal (2 dags) count matrix as the PE-stationary operand
and [E | E*y] as one 208-wide moving operand -> ONE matmul per dag-pair
per level.

Layout flips (B=[f,k] <-> A=[nodes,f]) are done with XBAR DMA
transposes (InstDmaTransposeAnt, 16x128 tiles) on the otherwise-idle
DMA engines instead of PE transposes. The 2F-contraction merge matmul
is batched 512 moving columns at a time (4 dag-pairs), so the PE runs
only 16 count matmuls + 8 merge matmuls per level. The softmax divide
uses the fast custom-DVE reciprocal (reciprocal_approx_fast).

State y^T = (att*out)/16 kept in fp16 B-layout [104(f) x 2048 (16
pairs x 128)] tiles; att_w and the /16 scaling are folded into the
weights on the host. atom_feats are pre-transposed/cast to fp16 on the
host.

Final sink softmax-pool: raw sink states are emitted per core and
reduced on the host, followed by the tiny [104]x[104,500] output
linear.
"""

import sys

sys.path.insert(0, "/opt/trn_rl_repo")

import numpy as np

import concourse.bacc as bacc
import concourse.bass as bass
import concourse.mybir as mybir
import concourse.tile as tile
from concourse.bass_utils import run_bass_kernel_spmd

D, L, K, P, F, C = 256, 64, 64, 8, 104, 500
NCORES = 8
DPC = D // NCORES          # 32 dags per core
NPAIR = DPC // 2           # 16 pair-tiles
NGRP = 4                   # pairs are processed 4 at a time (512 cols)
SCALE = 16.0               # state stored as y/16 (fp16 headroom for E*y)

F16 = mybir.dt.float16
F32 = mybir.dt.float32

_compiled = {}


def _host_prep(atom_feats, pred_idx, W1, b1, Wm, bm, att_w, dag_w):
    """Build per-core DMA-ready tensors (numpy only)."""
    att = att_w.astype(np.float64)
    # effective weights (att folding + 1/SCALE state scaling), see module doc
    w1_eff = (W1.astype(np.float64) * att[None, :] / SCALE).astype(np.float16)
    b1_eff = (b1.astype(np.float64) * att / SCALE).astype(np.float16)
    wtop = (Wm[:F].astype(np.float64) * att[None, :] / att[:, None]).astype(np.float16)
    wbot = (Wm[F:].astype(np.float64) * att[None, :] / SCALE).astype(np.float16)
    bm_eff = (bm.astype(np.float64) * att / SCALE).astype(np.float16)

    # count matrices: ct[d,l,j,k'] = #{p: pred_idx[d,l,k',p]==j}
    rows = np.arange(D * (L - 1) * K, dtype=np.int64).repeat(P) * K
    lin = rows + pred_idx.reshape(-1).astype(np.int64)
    ct = np.bincount(lin, minlength=D * (L - 1) * K * K).astype(np.float16)
    ct = ct.reshape(D, L - 1, K, K)                                # [d,l,k',j]
    ct = np.swapaxes(ct, 2, 3)                                     # [d,l,j,k']

    # atomsT: [core, level, 104, NPAIR*128] fp16
    at = np.swapaxes(atom_feats, 2, 3).astype(np.float16)          # [d,l,f,k]
    at = at.reshape(NCORES, DPC, L, F, K)

    per_core = []
    for c in range(NCORES):
        a = at[c]                                                  # [32,64,104,64]
        a = a.reshape(NPAIR, 2, L, F, K)
        # [level, f, pair, dag-in-pair, k] -> [level, f, pair*128]
        a = a.transpose(2, 3, 0, 1, 4).reshape(L, F, NPAIR * 2 * K)
        atomsT = np.ascontiguousarray(a)                           # [64,104,2048]

        cc = ct.reshape(NCORES, DPC, L - 1, K, K)[c]               # [32,63,64,64]
        cc = cc.reshape(NPAIR, 2, L - 1, K, K)
        # block-diagonal stationary count matrices, one [128,128] per pair
        ctb = np.zeros((L - 1, 2 * K, NPAIR, 2 * K), np.float16)
        ctb[:, 0:K, :, 0:K] = cc[:, 0].transpose(1, 2, 0, 3)       # [l,j,pair,k']
        ctb[:, K:2 * K, :, K:2 * K] = cc[:, 1].transpose(1, 2, 0, 3)
        ctb = np.ascontiguousarray(ctb.reshape(L - 1, 2 * K, NPAIR * 2 * K))
        per_core.append({
            "atomsT": atomsT, "ctb": ctb,
            "w1": w1_eff, "wbot": np.ascontiguousarray(wbot),
            "wtop": np.ascontiguousarray(wtop),
            "b1v": b1_eff.astype(np.float32)[:, None],
            "bmv": bm_eff.astype(np.float32)[:, None],
        })
    return per_core


def _build_program(levels=L):
    nc = bacc.Bacc("TRN2", target_bir_lowering=False, debug=False,
                   num_devices=NCORES)

    d_atomsT = nc.dram_tensor("atomsT", [L, F, NPAIR * 128], F16,
                              kind="ExternalInput").ap()
    d_ctb = nc.dram_tensor("ctb", [L - 1, 128, NPAIR * 128], F16,
                           kind="ExternalInput").ap()
    d_w1 = nc.dram_tensor("w1", [F, F], F16, kind="ExternalInput").ap()
    d_wbot = nc.dram_tensor("wbot", [F, F], F16, kind="ExternalInput").ap()
    d_wtop = nc.dram_tensor("wtop", [F, F], F16, kind="ExternalInput").ap()
    d_b1v = nc.dram_tensor("b1v", [F, 1], F32, kind="ExternalInput").ap()
    d_bmv = nc.dram_tensor("bmv", [F, 1], F32, kind="ExternalInput").ap()
    d_out = nc.dram_tensor("sinks", [F, DPC], F32, kind="ExternalOutput").ap()

    GW = 512               # B-layout columns per group (4 pairs)
    YP = 112               # padded partition count of Y (mult of 16)

    with tile.TileContext(nc) as tc:
        with tc.tile_pool(name="pool", bufs=1) as pool, \
             tc.tile_pool(name="psum", space="PSUM", bufs=1) as psum:
            # constants / weights
            w1 = pool.tile([F, F], F16, tag="w1")
            wbot = pool.tile([F, F], F16, tag="wbot")
            wtop = pool.tile([F, F], F16, tag="wtop")
            b1v = pool.tile([F, 1], F32, tag="b1v")
            bmv = pool.tile([F, 1], F32, tag="bmv")
            nc.sync.dma_start(w1[:], d_w1)
            nc.sync.dma_start(wbot[:], d_wbot)
            nc.sync.dma_start(wtop[:], d_wtop)
            nc.sync.dma_start(b1v[:], d_b1v)
            nc.sync.dma_start(bmv[:], d_bmv)

            # ---- level 0: y0 = relu(W1_aug.T @ atoms0) ----
            a0 = pool.tile([F, NPAIR * 128], F16, tag="atoms", bufs=3)
            nc.sync.dma_start(a0[:], d_atomsT[0])
            Yp = pool.tile([YP, NPAIR * 128], F16, tag="Y", bufs=2)
            nc.gpsimd.memset(Yp[96:YP, :], 0)
            for g in range(NGRP):
                z = psum.tile([F, GW], F32, tag="z", bufs=2)
                nc.tensor.matmul(z[:], w1[:], a0[:, GW * g:GW * (g + 1)],
                                 start=True, stop=True)
                nc.scalar.activation(Yp[0:F, GW * g:GW * (g + 1)], z[:],
                                     mybir.ActivationFunctionType.Relu,
                                     bias=b1v[:])

            # ---- levels 1..63 ----
            for lvl in range(1, levels):
                ctl = pool.tile([128, NPAIR * 128], F16, tag="ct", bufs=3)
                nc.sync.dma_start(ctl[:], d_ctb[lvl - 1])
                al = pool.tile([F, NPAIR * 128], F16, tag="atoms", bufs=3)
                nc.sync.dma_start(al[:], d_atomsT[lvl])

                YA = pool.tile([128, NPAIR * YP], F16, tag="YA", bufs=2)
                EEX = pool.tile([128, NPAIR * 2 * F], F16, tag="EEX", bufs=2)
                AGA = pool.tile([128, NPAIR * 128], F16, tag="AGA", bufs=2)
                AGB = pool.tile([128, NPAIR * 128], F16, tag="AGB", bufs=2)
                Y = pool.tile([YP, NPAIR * 128], F16, tag="Y", bufs=2)
                nc.gpsimd.memset(Y[96:YP, :], 0)
                nc.gpsimd.memset(
                    AGA[:].rearrange("p (t c) -> p t c", c=128)[:, :, F:128], 0)

                agbs = []
                for g in range(NGRP):
                    # XBAR transpose: 4 pair-tiles y^T -> node-major layout
                    ya3 = YA[:, 4 * YP * g:4 * YP * (g + 1)] \
                        .rearrange("p (t c) -> p t c", c=YP)
                    nc.sync.dma_start_transpose(
                        ya3, Yp[0:YP, GW * g:GW * (g + 1)])
                    eex_g = EEX[:, 8 * F * g:8 * F * (g + 1)] \
                        .rearrange("p (t c) -> p t c", c=2 * F)
                    nc.scalar.activation(eex_g[:, :, 0:F], ya3[:, :, 0:F],
                                         mybir.ActivationFunctionType.Exp,
                                         scale=SCALE)
                    nc.vector.tensor_tensor(eex_g[:, :, F:2 * F],
                                            eex_g[:, :, 0:F], ya3[:, :, 0:F],
                                            op=mybir.AluOpType.mult)
                    # den|num for 4 pairs: C blockdiag stationary,
                    # [E | E*y] moving (208 cols per pair)
                    dn = psum.tile([128, 1024], F32, tag="dn", bufs=2)
                    for i in range(4):
                        t = 4 * g + i
                        nc.tensor.matmul(dn[:, 256 * i:256 * i + 2 * F],
                                         ctl[:, 128 * t:128 * (t + 1)],
                                         EEX[:, 2 * F * t:2 * F * (t + 1)],
                                         start=True, stop=True)
                    dn3 = dn[:].rearrange("p (t c) -> p t c", c=256)
                    rd = pool.tile([128, 4 * F], F32, tag="rd", bufs=2)
                    rd3 = rd[:].rearrange("p (t c) -> p t c", c=F)
                    nc.vector.reciprocal_approx_fast(rd3, dn3[:, :, 0:F])
                    aga3 = AGA[:, GW * g:GW * (g + 1)] \
                        .rearrange("p (t c) -> p t c", c=128)
                    nc.vector.tensor_tensor(aga3[:, :, 0:F],
                                            dn3[:, :, F:2 * F], rd3,
                                            op=mybir.AluOpType.mult)
                    # XBAR transpose back to [f, k] for the merge matmul
                    agb3 = AGB[:, GW * g:GW * (g + 1)] \
                        .rearrange("p (t c) -> p t c", c=128)
                    nc.sync.dma_start_transpose(
                        agb3, AGA[:, GW * g:GW * (g + 1)])
                    agbs.append(g)

                for g in agbs:
                    z = psum.tile([F, GW], F32, tag="z", bufs=2)
                    nc.tensor.matmul(z[:], wtop[:],
                                     AGB[0:F, GW * g:GW * (g + 1)],
                                     start=True, stop=False)
                    nc.tensor.matmul(z[:], wbot[:],
                                     al[:, GW * g:GW * (g + 1)],
                                     start=False, stop=True)
                    nc.scalar.activation(Y[0:F, GW * g:GW * (g + 1)], z[:],
                                         mybir.ActivationFunctionType.Relu,
                                         bias=bmv[:])
                Yp = Y

            # ---- final: emit raw sink state (y/16 at node K-1) [F, DPC] ----
            sk = pool.tile([F, DPC], F32, tag="sk")
            nc.scalar.copy(
                sk[:],
                Yp[0:F].rearrange("p (d k) -> p d k", k=K)[:, :, K - 1])
            nc.sync.dma_start(d_out, sk[:])

    nc.compile()
    return nc


def kernel(atom_feats, pred_idx, W1, b1, Wm, bm, att_w, dag_w, Wf, bf):
    atom_feats = np.asarray(atom_feats, np.float32)
    pred_idx = np.asarray(pred_idx, np.int32)
    per_core = _host_prep(atom_feats, pred_idx,
                          np.asarray(W1, np.float32), np.asarray(b1, np.float32),
                          np.asarray(Wm, np.float32), np.asarray(bm, np.float32),
                          np.asarray(att_w, np.float32), np.asarray(dag_w, np.float32))

    if "nc" not in _compiled:
        _compiled["nc"] = _build_program()
    nc = _compiled["nc"]

    import os
    in_maps = [{k: v for k, v in pc.items()} for pc in per_core]
    trace = bool(os.environ.get("BASS_KERNEL_TRACE"))
    res = run_bass_kernel_spmd(nc, in_maps, list(range(NCORES)), trace=trace)
    _compiled["exec_time_ns"] = res.exec_time_ns

    att = np.asarray(att_w, np.float64)[:, None]
    dagw = np.asarray(dag_w, np.float64)[:, None]
    sinks = np.concatenate(
        [np.asarray(r["sinks"], np.float64) for r in res.results], axis=1)
    sink = sinks * SCALE / att                     # [F, D] true sink values
    u = np.exp(dagw * sink)
    pooled = (u * sink).sum(1) / u.sum(1)
    out = pooled @ np.asarray(Wf, np.float64) + np.asarray(bf, np.float64)
    return out.astype(np.float32)


# revision 6
# speedup vs baseline: 1.2966x; 1.2618x over previous
"""Trainium2 Bass kernel for nn_ChEBIRecNN (gnn_message_passing).

Strategy
--------
D=256 DAGs sharded 32/core across 8 NeuronCores (data parallel).

The per-level softmax-attention gather is reformulated with predecessor
COUNT matrices (host-precomputed from pred_idx):
    C_d[j,k'] = #{p : pred_idx[d,l,k',p] == j}
    den[f,k'] = sum_j E[j,f] * C[j,k'],   E = exp(att*out)
    num[f,k'] = sum_j (E*y)[j,f] * C[j,k']
    agg       = num / den
turning gather+softmax+reduce into dense 128-contraction matmuls with
E / E*y as the PE-stationary operand and the block-diagonal (2 dags)
count matrix moving, so den/num land directly in [f, k] layout for the
merge matmul (no second transpose).

The single per-level layout flip (state [f,k] -> node-major [k,f] for
exp) is done with XBAR DMA transposes (InstDmaTransposeAnt) on the
otherwise-idle DMA engines, one per 4-pair group to keep the
cross-level dependency chain fine-grained. The 2F-contraction merge
matmul is batched 512 moving columns (4 dag-pairs) per instruction.
The softmax divide uses the fast custom-DVE reciprocal
(reciprocal_approx_fast), and exp / E*y / reciprocal / divide are all
batched per group.

State y^T = (att*out)/16 kept in fp16 [104(f) x 2048 (16 pairs x 128)]
tiles; att_w and the /16 scaling are folded into the weights on the
host. atom_feats are pre-transposed/cast to fp16 on the host.

Final sink softmax-pool: raw sink states are emitted per core and
reduced on the host, followed by the tiny [104]x[104,500] output
linear.
"""

import sys

sys.path.insert(0, "/opt/trn_rl_repo")

import numpy as np

import concourse.bacc as bacc
import concourse.bass as bass
import concourse.mybir as mybir
import concourse.tile as tile
from concourse.bass_utils import run_bass_kernel_spmd

D, L, K, P, F, C = 256, 64, 64, 8, 104, 500
NCORES = 8
DPC = D // NCORES          # 32 dags per core
NPAIR = DPC // 2           # 16 pair-tiles
NGRP = 4                   # pairs are processed 4 at a time (512 cols)
SCALE = 16.0               # state stored as y/16 (fp16 headroom for E*y)

F16 = mybir.dt.float16
F32 = mybir.dt.float32

_compiled = {}


def _host_prep(atom_feats, pred_idx, W1, b1, Wm, bm, att_w, dag_w):
    """Build per-core DMA-ready tensors (numpy only)."""
    att = att_w.astype(np.float64)
    # effective weights (att folding + 1/SCALE state scaling), see module doc
    w1_eff = (W1.astype(np.float64) * att[None, :] / SCALE).astype(np.float16)
    b1_eff = (b1.astype(np.float64) * att / SCALE).astype(np.float16)
    wtop = (Wm[:F].astype(np.float64) * att[None, :] / att[:, None]).astype(np.float16)
    wbot = (Wm[F:].astype(np.float64) * att[None, :] / SCALE).astype(np.float16)
    bm_eff = (bm.astype(np.float64) * att / SCALE).astype(np.float16)

    # count matrices: ct[d,l,j,k'] = #{p: pred_idx[d,l,k',p]==j}
    rows = np.arange(D * (L - 1) * K, dtype=np.int64).repeat(P) * K
    lin = rows + pred_idx.reshape(-1).astype(np.int64)
    ct = np.bincount(lin, minlength=D * (L - 1) * K * K).astype(np.float16)
    ct = ct.reshape(D, L - 1, K, K)                                # [d,l,k',j]
    ct = np.swapaxes(ct, 2, 3)                                     # [d,l,j,k']

    # atomsT: [core, level, 104, NPAIR*128] fp16
    at = np.swapaxes(atom_feats, 2, 3).astype(np.float16)          # [d,l,f,k]
    at = at.reshape(NCORES, DPC, L, F, K)

    per_core = []
    for c in range(NCORES):
        a = at[c]                                                  # [32,64,104,64]
        a = a.reshape(NPAIR, 2, L, F, K)
        # [level, f, pair, dag-in-pair, k] -> [level, f, pair*128]
        a = a.transpose(2, 3, 0, 1, 4).reshape(L, F, NPAIR * 2 * K)
        atomsT = np.ascontiguousarray(a)                           # [64,104,2048]

        cc = ct.reshape(NCORES, DPC, L - 1, K, K)[c]               # [32,63,64,64]
        cc = cc.reshape(NPAIR, 2, L - 1, K, K)
        # block-diagonal moving count matrices, one [128,128] per pair
        ctb = np.zeros((L - 1, 2 * K, NPAIR, 2 * K), np.float16)
        ctb[:, 0:K, :, 0:K] = cc[:, 0].transpose(1, 2, 0, 3)       # [l,j,pair,k']
        ctb[:, K:2 * K, :, K:2 * K] = cc[:, 1].transpose(1, 2, 0, 3)
        ctb = np.ascontiguousarray(ctb.reshape(L - 1, 2 * K, NPAIR * 2 * K))
        per_core.append({
            "atomsT": atomsT, "ctb": ctb,
            "w1": w1_eff, "wbot": np.ascontiguousarray(wbot),
            "wtop": np.ascontiguousarray(wtop),
            "b1v": b1_eff.astype(np.float32)[:, None],
            "bmv": bm_eff.astype(np.float32)[:, None],
        })
    return per_core


def _build_program(levels=L):
    nc = bacc.Bacc("TRN2", target_bir_lowering=False, debug=False,
                   num_devices=NCORES)

    d_atomsT = nc.dram_tensor("atomsT", [L, F, NPAIR * 128], F16,
                              kind="ExternalInput").ap()
    d_ctb = nc.dram_tensor("ctb", [L - 1, 128, NPAIR * 128], F16,
                           kind="ExternalInput").ap()
    d_w1 = nc.dram_tensor("w1", [F, F], F16, kind="ExternalInput").ap()
    d_wbot = nc.dram_tensor("wbot", [F, F], F16, kind="ExternalInput").ap()
    d_wtop = nc.dram_tensor("wtop", [F, F], F16, kind="ExternalInput").ap()
    d_b1v = nc.dram_tensor("b1v", [F, 1], F32, kind="ExternalInput").ap()
    d_bmv = nc.dram_tensor("bmv", [F, 1], F32, kind="ExternalInput").ap()
    d_out = nc.dram_tensor("sinks", [F, DPC], F32, kind="ExternalOutput").ap()

    GW = 512               # B-layout columns per group (4 pairs)
    YP = 112               # padded partition count of Y (mult of 16)

    with tile.TileContext(nc) as tc:
        with tc.tile_pool(name="pool", bufs=1) as pool, \
             tc.tile_pool(name="psum", space="PSUM", bufs=1) as psum:
            # constants / weights
            w1 = pool.tile([F, F], F16, tag="w1")
            wbot = pool.tile([F, F], F16, tag="wbot")
            wtop = pool.tile([F, F], F16, tag="wtop")
            b1v = pool.tile([F, 1], F32, tag="b1v")
            bmv = pool.tile([F, 1], F32, tag="bmv")
            nc.sync.dma_start(w1[:], d_w1)
            nc.sync.dma_start(wbot[:], d_wbot)
            nc.sync.dma_start(wtop[:], d_wtop)
            nc.sync.dma_start(b1v[:], d_b1v)
            nc.sync.dma_start(bmv[:], d_bmv)

            # ---- level 0: y0 = relu(W1_aug.T @ atoms0) ----
            a0 = pool.tile([F, NPAIR * 128], F16, tag="atoms", bufs=3)
            nc.scalar.dma_start(a0[:], d_atomsT[0])
            Yp = pool.tile([YP, NPAIR * 128], F16, tag="Y", bufs=2)
            nc.gpsimd.memset(Yp[96:YP, :], 0)
            for g in range(NGRP):
                z = psum.tile([F, GW], F32, tag="z", bufs=2)
                nc.tensor.matmul(z[:], w1[:], a0[:, GW * g:GW * (g + 1)],
                                 start=True, stop=True)
                nc.scalar.activation(Yp[0:F, GW * g:GW * (g + 1)], z[:],
                                     mybir.ActivationFunctionType.Relu,
                                     bias=b1v[:])

            # ---- levels 1..63 ----
            for lvl in range(1, levels):
                ctl = pool.tile([128, NPAIR * 128], F16, tag="ct", bufs=3)
                nc.sync.dma_start(ctl[:], d_ctb[lvl - 1])
                al = pool.tile([F, NPAIR * 128], F16, tag="atoms", bufs=3)
                nc.scalar.dma_start(al[:], d_atomsT[lvl])

                YA = pool.tile([128, NPAIR * YP], F16, tag="YA", bufs=2)
                EEX = pool.tile([128, NPAIR * 2 * F], F16, tag="EEX", bufs=2)
                AG = pool.tile([F, NPAIR * 128], F16, tag="AG", bufs=2)
                Y = pool.tile([YP, NPAIR * 128], F16, tag="Y", bufs=2)
                nc.gpsimd.memset(Y[96:YP, :], 0)

                for g in range(NGRP):
                    # XBAR transpose: 4 pair-tiles y^T -> node-major layout
                    ya3 = YA[:, 4 * YP * g:4 * YP * (g + 1)] \
                        .rearrange("p (t c) -> p t c", c=YP)
                    nc.sync.dma_start_transpose(
                        ya3, Yp[0:YP, GW * g:GW * (g + 1)])
                    eex_g = EEX[:, 8 * F * g:8 * F * (g + 1)] \
                        .rearrange("p (t c) -> p t c", c=2 * F)
                    nc.scalar.activation(eex_g[:, :, 0:F], ya3[:, :, 0:F],
                                         mybir.ActivationFunctionType.Exp,
                                         scale=SCALE)
                    nc.vector.tensor_tensor(eex_g[:, :, F:2 * F],
                                            eex_g[:, :, 0:F], ya3[:, :, 0:F],
                                            op=mybir.AluOpType.mult)
                    # den|num for 4 pairs: E / E*y stationary, C moving
                    # -> [f, k] layout directly
                    dn = psum.tile([F, 1024], F32, tag="dn", bufs=2)
                    for i in range(4):
                        t = 4 * g + i
                        nc.tensor.matmul(dn[:, 256 * i:256 * i + 128],
                                         EEX[:, 2 * F * t:2 * F * t + F],
                                         ctl[:, 128 * t:128 * (t + 1)],
                                         start=True, stop=True)
                        nc.tensor.matmul(dn[:, 256 * i + 128:256 * (i + 1)],
                                         EEX[:, 2 * F * t + F:2 * F * (t + 1)],
                                         ctl[:, 128 * t:128 * (t + 1)],
                                         start=True, stop=True)
                    dn3 = dn[:].rearrange("p (t c) -> p t c", c=256)
                    rd = pool.tile([F, GW], F32, tag="rd", bufs=2)
                    rd3 = rd[:].rearrange("p (t c) -> p t c", c=128)
                    nc.vector.reciprocal_approx_fast(rd3, dn3[:, :, 0:128])
                    ag3 = AG[:, GW * g:GW * (g + 1)] \
                        .rearrange("p (t c) -> p t c", c=128)
                    nc.vector.tensor_tensor(ag3, dn3[:, :, 128:256], rd3,
                                            op=mybir.AluOpType.mult)
                    # merge: cat(agg, atoms) @ Wm, batched over the group
                    z = psum.tile([F, GW], F32, tag="z", bufs=2)
                    nc.tensor.matmul(z[:], wtop[:],
                                     AG[:, GW * g:GW * (g + 1)],
                                     start=True, stop=False)
                    nc.tensor.matmul(z[:], wbot[:],
                                     al[:, GW * g:GW * (g + 1)],
                                     start=False, stop=True)
                    nc.scalar.activation(Y[0:F, GW * g:GW * (g + 1)], z[:],
                                         mybir.ActivationFunctionType.Relu,
                                         bias=bmv[:])
                Yp = Y

            # ---- final: emit raw sink state (y/16 at node K-1) [F, DPC] ----
            sk = pool.tile([F, DPC], F32, tag="sk")
            nc.scalar.copy(
                sk[:],
                Yp[0:F].rearrange("p (d k) -> p d k", k=K)[:, :, K - 1])
            nc.sync.dma_start(d_out, sk[:])

    nc.compile()
    return nc


def kernel(atom_feats, pred_idx, W1, b1, Wm, bm, att_w, dag_w, Wf, bf):
    atom_feats = np.asarray(atom_feats, np.float32)
    pred_idx = np.asarray(pred_idx, np.int32)
    per_core = _host_prep(atom_feats, pred_idx,
                          np.asarray(W1, np.float32), np.asarray(b1, np.float32),
                          np.asarray(Wm, np.float32), np.asarray(bm, np.float32),
                          np.asarray(att_w, np.float32), np.asarray(dag_w, np.float32))

    if "nc" not in _compiled:
        _compiled["nc"] = _build_program()
    nc = _compiled["nc"]

    import os
    in_maps = [{k: v for k, v in pc.items()} for pc in per_core]
    trace = bool(os.environ.get("BASS_KERNEL_TRACE"))
    res = run_bass_kernel_spmd(nc, in_maps, list(range(NCORES)), trace=trace)
    _compiled["exec_time_ns"] = res.exec_time_ns

    att = np.asarray(att_w, np.float64)[:, None]
    dagw = np.asarray(dag_w, np.float64)[:, None]
    sinks = np.concatenate(
        [np.asarray(r["sinks"], np.float64) for r in res.results], axis=1)
    sink = sinks * SCALE / att                     # [F, D] true sink values
    u = np.exp(dagw * sink)
    pooled = (u * sink).sum(1) / u.sum(1)
    out = pooled @ np.asarray(Wf, np.float64) + np.asarray(bf, np.float64)
    return out.astype(np.float32)


# revision 7
# speedup vs baseline: 1.4980x; 1.1553x over previous
"""Trainium2 Bass kernel for nn_ChEBIRecNN (gnn_message_passing).

Strategy
--------
D=256 DAGs sharded 32/core across 8 NeuronCores (data parallel).

The per-level softmax-attention gather is reformulated with predecessor
COUNT matrices (host-precomputed from pred_idx):
    C_d[j,k'] = #{p : pred_idx[d,l,k',p] == j}
    den[f,k'] = sum_j E[j,f] * C[j,k'],   E = exp(att*out)
    num[f,k'] = sum_j (E*y)[j,f] * C[j,k']
    agg       = num / den
turning gather+softmax+reduce into dense 128-contraction matmuls with
E / E*y as the PE-stationary operand and the block-diagonal (2 dags)
count matrix moving, so den/num land directly in [f, k] layout for the
merge matmul (no second transpose).

The single per-level layout flip (state [f,k] -> node-major [k,f] for
exp) is done with XBAR DMA transposes (InstDmaTransposeAnt) on the
otherwise-idle DMA engines, one per 4-pair group to keep the
cross-level dependency chain fine-grained. The 2F-contraction merge
matmul is batched 512 moving columns (4 dag-pairs) per instruction.
The softmax divide uses the fast custom-DVE reciprocal
(reciprocal_approx_fast), and exp / E*y / reciprocal / divide are all
batched per group.

State y^T = (att*out)/16 kept in fp16 [104(f) x 2048 (16 pairs x 128)]
tiles; att_w and the /16 scaling are folded into the weights on the
host. atom_feats are pre-transposed/cast to fp16 on the host.

Final sink softmax-pool: raw sink states are emitted per core and
reduced on the host, followed by the tiny [104]x[104,500] output
linear.
"""

import sys

sys.path.insert(0, "/opt/trn_rl_repo")

import numpy as np

import concourse.bacc as bacc
import concourse.bass as bass
import concourse.mybir as mybir
import concourse.tile as tile
from concourse.bass_utils import run_bass_kernel_spmd

D, L, K, P, F, C = 256, 64, 64, 8, 104, 500
NCORES = 8
DPC = D // NCORES          # 32 dags per core
NPAIR = DPC // 2           # 16 pair-tiles
NGRP = 4                   # pairs are processed 4 at a time (512 cols)
SCALE = 16.0               # state stored as y/16 (fp16 headroom for E*y)

F16 = mybir.dt.float16
F32 = mybir.dt.float32

_compiled = {}


def _host_prep(atom_feats, pred_idx, W1, b1, Wm, bm, att_w, dag_w):
    """Build per-core DMA-ready tensors (numpy only)."""
    att = att_w.astype(np.float64)
    # effective weights (att folding + 1/SCALE state scaling), see module doc
    w1_eff = (W1.astype(np.float64) * att[None, :] / SCALE).astype(np.float16)
    b1_eff = (b1.astype(np.float64) * att / SCALE).astype(np.float16)
    wtop = (Wm[:F].astype(np.float64) * att[None, :] / att[:, None]).astype(np.float16)
    wbot = (Wm[F:].astype(np.float64) * att[None, :] / SCALE).astype(np.float16)
    bm_eff = (bm.astype(np.float64) * att / SCALE).astype(np.float16)

    # count matrices: ct[d,l,j,k'] = #{p: pred_idx[d,l,k',p]==j}
    rows = np.arange(D * (L - 1) * K, dtype=np.int64).repeat(P) * K
    lin = rows + pred_idx.reshape(-1).astype(np.int64)
    ct = np.bincount(lin, minlength=D * (L - 1) * K * K).astype(np.float16)
    ct = ct.reshape(D, L - 1, K, K)                                # [d,l,k',j]
    ct = np.swapaxes(ct, 2, 3)                                     # [d,l,j,k']

    # atomsT: [core, level, 104, NPAIR*128] fp16
    at = np.swapaxes(atom_feats, 2, 3).astype(np.float16)          # [d,l,f,k]
    at = at.reshape(NCORES, DPC, L, F, K)

    per_core = []
    for c in range(NCORES):
        a = at[c]                                                  # [32,64,104,64]
        a = a.reshape(NPAIR, 2, L, F, K)
        # [level, f, pair, dag-in-pair, k] -> [level, f, pair*128]
        a = a.transpose(2, 3, 0, 1, 4).reshape(L, F, NPAIR * 2 * K)
        atomsT = np.ascontiguousarray(a)                           # [64,104,2048]

        cc = ct.reshape(NCORES, DPC, L - 1, K, K)[c]               # [32,63,64,64]
        cc = cc.reshape(NPAIR, 2, L - 1, K, K)
        # block-diagonal moving count matrices, one [128,128] per pair
        ctb = np.zeros((L - 1, 2 * K, NPAIR, 2 * K), np.float16)
        ctb[:, 0:K, :, 0:K] = cc[:, 0].transpose(1, 2, 0, 3)       # [l,j,pair,k']
        ctb[:, K:2 * K, :, K:2 * K] = cc[:, 1].transpose(1, 2, 0, 3)
        ctb = np.ascontiguousarray(ctb.reshape(L - 1, 2 * K, NPAIR * 2 * K))
        per_core.append({
            "atomsT": atomsT, "ctb": ctb,
            "w1": w1_eff, "wbot": np.ascontiguousarray(wbot),
            "wtop": np.ascontiguousarray(wtop),
            "b1v": b1_eff.astype(np.float32)[:, None],
            "bmv": bm_eff.astype(np.float32)[:, None],
        })
    return per_core


def _build_program(levels=L):
    nc = bacc.Bacc("TRN2", target_bir_lowering=False, debug=False,
                   num_devices=NCORES)

    d_atomsT = nc.dram_tensor("atomsT", [L, F, NPAIR * 128], F16,
                              kind="ExternalInput").ap()
    d_ctb = nc.dram_tensor("ctb", [L - 1, 128, NPAIR * 128], F16,
                           kind="ExternalInput").ap()
    d_w1 = nc.dram_tensor("w1", [F, F], F16, kind="ExternalInput").ap()
    d_wbot = nc.dram_tensor("wbot", [F, F], F16, kind="ExternalInput").ap()
    d_wtop = nc.dram_tensor("wtop", [F, F], F16, kind="ExternalInput").ap()
    d_b1v = nc.dram_tensor("b1v", [F, 1], F32, kind="ExternalInput").ap()
    d_bmv = nc.dram_tensor("bmv", [F, 1], F32, kind="ExternalInput").ap()
    d_out = nc.dram_tensor("sinks", [F, DPC], F32, kind="ExternalOutput").ap()

    GW = 512               # B-layout columns per group (4 pairs)
    YP = 112               # padded partition count of Y (mult of 16)

    with tile.TileContext(nc) as tc:
        with tc.tile_pool(name="pool", bufs=1) as pool, \
             tc.tile_pool(name="psum", space="PSUM", bufs=1) as psum:
            # constants / weights
            w1 = pool.tile([F, F], F16, tag="w1")
            wbot = pool.tile([F, F], F16, tag="wbot")
            wtop = pool.tile([F, F], F16, tag="wtop")
            b1v = pool.tile([F, 1], F32, tag="b1v")
            bmv = pool.tile([F, 1], F32, tag="bmv")
            nc.sync.dma_start(w1[:], d_w1)
            nc.sync.dma_start(wbot[:], d_wbot)
            nc.sync.dma_start(wtop[:], d_wtop)
            nc.sync.dma_start(b1v[:], d_b1v)
            nc.sync.dma_start(bmv[:], d_bmv)

            # ---- level 0: y0 = relu(W1_aug.T @ atoms0) ----
            a0 = pool.tile([F, NPAIR * 128], F16, tag="atoms", bufs=3)
            nc.scalar.dma_start(a0[:], d_atomsT[0])
            Yp = pool.tile([YP, NPAIR * 128], F16, tag="Y", bufs=2)
            nc.gpsimd.memset(Yp[96:YP, :], 0)
            for g in range(NGRP):
                z = psum.tile([F, GW], F32, tag="z", bufs=2)
                nc.tensor.matmul(z[:], w1[:], a0[:, GW * g:GW * (g + 1)],
                                 start=True, stop=True)
                nc.scalar.activation(Yp[0:F, GW * g:GW * (g + 1)], z[:],
                                     mybir.ActivationFunctionType.Relu,
                                     bias=b1v[:])

            # ---- levels 1..63 ----
            for lvl in range(1, levels):
                ctl = pool.tile([128, NPAIR * 128], F16, tag="ct", bufs=3)
                nc.scalar.dma_start(ctl[:], d_ctb[lvl - 1])
                al = pool.tile([F, NPAIR * 128], F16, tag="atoms", bufs=3)
                nc.scalar.dma_start(al[:], d_atomsT[lvl])

                YA = pool.tile([128, NPAIR * YP], F16, tag="YA", bufs=2)
                EEX = pool.tile([128, NPAIR * 2 * F], F16, tag="EEX", bufs=2)
                AG = pool.tile([F, NPAIR * 128], F16, tag="AG", bufs=2)
                Y = pool.tile([YP, NPAIR * 128], F16, tag="Y", bufs=2)
                nc.gpsimd.memset(Y[96:YP, :], 0)

                # XBAR transposes: 8 pair-tiles y^T -> node-major per instr
                for h in range(2):
                    yh3 = YA[:, 8 * YP * h:8 * YP * (h + 1)] \
                        .rearrange("p (t c) -> p t c", c=YP)
                    nc.sync.dma_start_transpose(
                        yh3, Yp[0:YP, 2 * GW * h:2 * GW * (h + 1)])

                for g in range(NGRP):
                    ya3 = YA[:, 4 * YP * g:4 * YP * (g + 1)] \
                        .rearrange("p (t c) -> p t c", c=YP)
                    eex_g = EEX[:, 8 * F * g:8 * F * (g + 1)] \
                        .rearrange("p (t c) -> p t c", c=2 * F)
                    nc.scalar.activation(eex_g[:, :, 0:F], ya3[:, :, 0:F],
                                         mybir.ActivationFunctionType.Exp,
                                         scale=SCALE)
                    nc.vector.tensor_tensor(eex_g[:, :, F:2 * F],
                                            eex_g[:, :, 0:F], ya3[:, :, 0:F],
                                            op=mybir.AluOpType.mult)
                    # den|num for 4 pairs: E / E*y stationary, C moving
                    # -> [f, k] layout directly
                    dn = psum.tile([F, 1024], F32, tag="dn", bufs=2)
                    for i in range(4):
                        t = 4 * g + i
                        nc.tensor.matmul(dn[:, 256 * i:256 * i + 128],
                                         EEX[:, 2 * F * t:2 * F * t + F],
                                         ctl[:, 128 * t:128 * (t + 1)],
                                         start=True, stop=True)
                        nc.tensor.matmul(dn[:, 256 * i + 128:256 * (i + 1)],
                                         EEX[:, 2 * F * t + F:2 * F * (t + 1)],
                                         ctl[:, 128 * t:128 * (t + 1)],
                                         start=True, stop=True)
                    dn3 = dn[:].rearrange("p (t c) -> p t c", c=256)
                    rd = pool.tile([F, GW], F32, tag="rd", bufs=2)
                    rd3 = rd[:].rearrange("p (t c) -> p t c", c=128)
                    nc.vector.reciprocal_approx_fast(rd3, dn3[:, :, 0:128])
                    ag3 = AG[:, GW * g:GW * (g + 1)] \
                        .rearrange("p (t c) -> p t c", c=128)
                    nc.vector.tensor_tensor(ag3, dn3[:, :, 128:256], rd3,
                                            op=mybir.AluOpType.mult)
                    # merge: cat(agg, atoms) @ Wm, batched over the group
                    z = psum.tile([F, GW], F32, tag="z", bufs=2)
                    nc.tensor.matmul(z[:], wtop[:],
                                     AG[:, GW * g:GW * (g + 1)],
                                     start=True, stop=False)
                    nc.tensor.matmul(z[:], wbot[:],
                                     al[:, GW * g:GW * (g + 1)],
                                     start=False, stop=True)
                    nc.scalar.activation(Y[0:F, GW * g:GW * (g + 1)], z[:],
                                         mybir.ActivationFunctionType.Relu,
                                         bias=bmv[:])
                Yp = Y

            # ---- final: emit raw sink state (y/16 at node K-1) [F, DPC] ----
            sk = pool.tile([F, DPC], F32, tag="sk")
            nc.scalar.copy(
                sk[:],
                Yp[0:F].rearrange("p (d k) -> p d k", k=K)[:, :, K - 1])
            nc.sync.dma_start(d_out, sk[:])

    nc.compile()
    return nc


def kernel(atom_feats, pred_idx, W1, b1, Wm, bm, att_w, dag_w, Wf, bf):
    atom_feats = np.asarray(atom_feats, np.float32)
    pred_idx = np.asarray(pred_idx, np.int32)
    per_core = _host_prep(atom_feats, pred_idx,
                          np.asarray(W1, np.float32), np.asarray(b1, np.float32),
                          np.asarray(Wm, np.float32), np.asarray(bm, np.float32),
                          np.asarray(att_w, np.float32), np.asarray(dag_w, np.float32))

    if "nc" not in _compiled:
        _compiled["nc"] = _build_program()
    nc = _compiled["nc"]

    import os
    in_maps = [{k: v for k, v in pc.items()} for pc in per_core]
    trace = bool(os.environ.get("BASS_KERNEL_TRACE"))
    res = run_bass_kernel_spmd(nc, in_maps, list(range(NCORES)), trace=trace)
    _compiled["exec_time_ns"] = res.exec_time_ns

    att = np.asarray(att_w, np.float64)[:, None]
    dagw = np.asarray(dag_w, np.float64)[:, None]
    sinks = np.concatenate(
        [np.asarray(r["sinks"], np.float64) for r in res.results], axis=1)
    sink = sinks * SCALE / att                     # [F, D] true sink values
    u = np.exp(dagw * sink)
    pooled = (u * sink).sum(1) / u.sum(1)
    out = pooled @ np.asarray(Wf, np.float64) + np.asarray(bf, np.float64)
    return out.astype(np.float32)
